# revision 1
# baseline (speedup 1.0000x reference)
"""Multi-head attention with exclusive post-processing, sharded over 8 trn2 cores.

Sharding: data-parallel over batch (2) x tensor-parallel over heads (16 -> 4/core).
Each core computes a partial transposed output [D, S] for its batch from its 4
heads; the host sums the 4 partials per batch, transposes back, and adds bo.

Device layouts are feature-major ("T" = [feature, position]) so every matmul
contraction sits on the partition axis:
  QT/KT [256, S]     <- W.T @ x.T  (bf16, head pairs stacked on partitions)
  v^T   [64, S]/head (base partition 0 so DVE ops stay partition-aligned)
  scoresT [keys, q]  <- KT_h slices.T @ QT_h
  P^T = exp(scoresT/8)   (ScalarE, scale folded into the activation)
  Y'[128, q] <- [V_h | ones].T @ P^T : rows 0..63 = unnormalized Y, rows
  64..127 = softmax denominator broadcast across partitions for free.
  Exclusive step in closed form: y_excl = (Y - (Y.v)/(sum v^2 + eps) v)/denom,
  with both reciprocals computed as exp(-ln(x)) on ScalarE (ln and exp share
  one ACT table set; DVE's iterative RECIPROCAL is ~8x slower).
  out^T[D, S] <- Wo_h.T slices @ y_excl (bf16, per-head K=64 contraction).

Phase D is split: D1 keeps PE/ACT dense (only a PSUM->SBUF copy and a Ln per
head leave the kc loop); D2 does the exclusive tail off the PE critical path,
interleaving with the next q-block's D1 and the out-projection.
"""

import os
from contextlib import ExitStack

import ml_dtypes
import numpy as np

import concourse.bass as bass
import concourse.mybir as mybir
import concourse.tile as tile
from concourse import bacc, bass_utils
from concourse.alu_op_type import AluOpType
from concourse.bass_isa import ReduceOp

F32 = mybir.dt.float32
F32R = mybir.dt.float32r
BF16 = mybir.dt.bfloat16
AF = mybir.ActivationFunctionType

B, S_FULL, D_FULL, H_FULL = 2, 2048, 1024, 16
HD = 64
N_CORES = 8
HEADS_PER_CORE = H_FULL * B // N_CORES  # 4


def build_nc(S=S_FULL, D=D_FULL, HL=HEADS_PER_CORE, use_bias=False):
    """Build the per-core Bass kernel. Returns a finalized Bacc object."""
    P = 128
    nH = HL * HD          # local fused head dim (256)
    KC = D // P           # x contraction chunks (8)
    NKc = S // P          # key chunks (16)
    QB = min(1024, S)     # q block (PSUM-sized)
    NQ = S // QB
    MT = nH // P          # feature M-tiles for QT/KT (2)
    DM = D // P           # out-proj M-tiles (8)
    NS = min(512, QB)     # matmul moving-dim chunk

    assert S % P == 0 and D % P == 0 and nH % P == 0 and QB % NS == 0

    _ensure_act_root()
    nc = bacc.Bacc(None, target_bir_lowering=False)

    xT_d = nc.dram_tensor("xT", [D, S], BF16, kind="ExternalInput")
    wq_d = nc.dram_tensor("wq", [D, nH], BF16, kind="ExternalInput")
    wk_d = nc.dram_tensor("wk", [D, nH], BF16, kind="ExternalInput")
    wv_d = nc.dram_tensor("wv", [D, nH], BF16, kind="ExternalInput")
    wo_d = nc.dram_tensor("wo", [nH, D], BF16, kind="ExternalInput")
    if use_bias:
        bq_d = nc.dram_tensor("bq", [1, nH], F32, kind="ExternalInput")
        bk_d = nc.dram_tensor("bk", [1, nH], F32, kind="ExternalInput")
        bv_d = nc.dram_tensor("bv", [1, nH], F32, kind="ExternalInput")
    outT_d = nc.dram_tensor("outT", [D, S], F32, kind="ExternalOutput")

    with tile.TileContext(nc) as tc, ExitStack() as ctx:
        consts = ctx.enter_context(tc.tile_pool(name="consts", bufs=1))
        psA = ctx.enter_context(tc.tile_pool(name="psA", bufs=2, space="PSUM"))
        psB = ctx.enter_context(tc.tile_pool(name="psB", bufs=2, space="PSUM"))
        pP = ctx.enter_context(tc.tile_pool(name="pP", bufs=4))
        ostgp = ctx.enter_context(tc.tile_pool(name="ostgp", bufs=2))
        stk = ctx.enter_context(tc.tile_pool(name="stk", bufs=2))
        bcs = ctx.enter_context(tc.tile_pool(name="bcs", bufs=2))
        bcs2 = ctx.enter_context(tc.tile_pool(name="bcs2", bufs=2))
        tps = ctx.enter_context(tc.tile_pool(name="tps", bufs=2))
        tps2 = ctx.enter_context(tc.tile_pool(name="tps2", bufs=2))
        ysbp = ctx.enter_context(tc.tile_pool(name="ysbp", bufs=6))
        lndp = ctx.enter_context(tc.tile_pool(name="lndp", bufs=5))

        # ---- ACT table preload: dummy exp+ln force the (single) table-set
        # load at kernel start, not as a 2.7us PE-stalling hiccup at the
        # start of the attention phase (which re-throttles the PE clock).
        smallc = consts.tile([P, 33], F32, tag="smallc")
        warm = smallc[0:1, 1:33]
        nc.vector.memset(warm, 1.0)
        nc.scalar.activation(out=warm, in_=warm, func=AF.Exp)
        nc.scalar.activation(out=warm, in_=warm, func=AF.Ln)

        # ---- input staging ----
        xT_sb = []
        for kc in range(KC):
            t = consts.tile([P, S], BF16, tag=f"xT{kc}")
            nc.sync.dma_start(out=t, in_=xT_d.ap()[kc * P:(kc + 1) * P, :])
            xT_sb.append(t)

        def load_w(dram):
            tiles = []
            for kc in range(KC):
                t = consts.tile([P, nH], BF16, tag=f"w{dram.name}{kc}")
                nc.sync.dma_start(out=t, in_=dram.ap()[kc * P:(kc + 1) * P, :])
                tiles.append(t)
            return tiles

        wq_sb, wk_sb, wv_sb = load_w(wq_d), load_w(wk_d), load_w(wv_d)

        wo_bf = []
        for h in range(HL):
            wbf = consts.tile([HD, D], BF16, tag=f"wobf_{h}", name=f"wobf_{h}")
            nc.sync.dma_start(out=wbf, in_=wo_d.ap()[h * HD:(h + 1) * HD, :])
            wo_bf.append(wbf)

        if use_bias:
            ones_row = consts.tile([1, max(S, P)], F32, tag="ones_row")
            nc.vector.memset(ones_row, 1.0)
            b_sb = {}
            for name, dram in (("q", bq_d), ("k", bk_d), ("v", bv_d)):
                t = consts.tile([1, nH], F32, tag=f"b{name}")
                nc.sync.dma_start(out=t, in_=dram.ap())
                b_sb[name] = t

        # eps vector for the ln(sum v^2 + eps) bias
        epsv = smallc[:, 0:1]
        nc.vector.memset(epsv, 1e-12)
        # ones64x64: all-ones [64,64] -> column-sum matmuls produce the result
        # broadcast across all 64 output partitions for free
        ones64x64 = consts.tile([HD, HD], BF16, tag="ones64x64")
        nc.vector.memset(ones64x64, 1.0)

        # ---- phase B: feature-major projections QT/KT [nH, S] (bf16, head pairs) ----
        QT = [consts.tile([P, S], BF16, tag=f"QT{t_i}", name=f"QT{t_i}") for t_i in range(MT)]
        KT = [consts.tile([P, S], BF16, tag=f"KT{t_i}", name=f"KT{t_i}") for t_i in range(MT)]

        def emit_qk(mt):
            for w_sb, dst, bias_key in ((wq_sb, QT, "q"), (wk_sb, KT, "k")):
                for qb in range(NQ):
                    ps = psA.tile([P, QB], F32, tag="ps", name="ps_qk")
                    if use_bias:
                        for ns in range(0, QB, NS):
                            nc.tensor.matmul(
                                ps[:, ns:ns + NS],
                                lhsT=b_sb[bias_key][:, mt * P:(mt + 1) * P].bitcast(F32R),
                                rhs=ones_row[:, :NS].bitcast(F32R),
                                start=True, stop=False)
                    for kc in range(KC):
                        for ns in range(0, QB, NS):
                            nc.tensor.matmul(
                                ps[:, ns:ns + NS],
                                lhsT=w_sb[kc][:, mt * P:(mt + 1) * P],
                                rhs=xT_sb[kc][:, qb * QB + ns:qb * QB + ns + NS],
                                start=(kc == 0 and not use_bias), stop=(kc == KC - 1))
                    nc.vector.tensor_copy(out=dst[mt][:, qb * QB:(qb + 1) * QB], in_=ps)

        # ---- phase B2: per-head v^T [64, S] at base partition 0 (DVE alignment) ----
        VTh = [consts.tile([HD, S], BF16, tag=f"VTh{h}", name=f"VTh{h}") for h in range(HL)]

        def emit_vth(h):
            for qb in range(NQ):
                ps = psA.tile([HD, QB], F32, tag="ps", name="ps_vth")
                if use_bias:
                    for ns in range(0, QB, NS):
                        nc.tensor.matmul(
                            ps[:, ns:ns + NS],
                            lhsT=b_sb["v"][:, h * HD:(h + 1) * HD].bitcast(F32R),
                            rhs=ones_row[:, :NS].bitcast(F32R),
                            start=True, stop=False)
                for kc in range(KC):
                    for ns in range(0, QB, NS):
                        nc.tensor.matmul(
                            ps[:, ns:ns + NS],
                            lhsT=wv_sb[kc][:, h * HD:(h + 1) * HD],
                            rhs=xT_sb[kc][:, qb * QB + ns:qb * QB + ns + NS],
                            start=(kc == 0 and not use_bias), stop=(kc == KC - 1))
                nc.vector.tensor_copy(out=VTh[h][:, qb * QB:(qb + 1) * QB], in_=ps)

        # ---- phase C: position-major V with a 64-wide ones block ----
        vprime = consts.tile([P, NKc, HL, 2 * HD], BF16, tag="vprime")

        def emit_vprime():
            nc.vector.memset(vprime[:, :, :, HD:2 * HD], 1.0)
            for qt in range(NKc):
                ps = psA.tile([P, nH], F32, tag="ps", name="ps_vp")
                if use_bias:
                    nc.tensor.matmul(
                        ps, lhsT=ones_row[:, 0:P].bitcast(F32R),
                        rhs=b_sb["v"].bitcast(F32R), start=True, stop=False)
                for kc in range(KC):
                    nc.tensor.matmul(
                        ps,
                        lhsT=xT_sb[kc][:, qt * P:(qt + 1) * P],
                        rhs=wv_sb[kc],
                        start=(kc == 0 and not use_bias), stop=(kc == KC - 1))
                nc.vector.tensor_copy(
                    out=vprime[:, qt, :, 0:HD],
                    in_=ps.rearrange("p (h d) -> p h d", h=HL))

        def head_slice(tiles, h):
            return tiles[h // 2][64 * (h % 2):64 * (h % 2) + 64, :]

        y_excl = [consts.tile([HD, S], BF16, tag=f"yx{h}", name=f"yx{h}") for h in range(HL)]

        def emit_d1(qb, h):
            q0 = qb * QB
            KTh, QTh = (head_slice(t, h) for t in (KT, QT))
            yp = psB.tile([P, QB], F32, tag="yp", name=f"yp{h}")

            def attn_v(pT, kc):
                for ns in range(0, QB, NS):
                    nc.tensor.matmul(
                        yp[:, ns:ns + NS],
                        lhsT=vprime[:, kc, h, :],
                        rhs=pT[:, ns:ns + NS],
                        start=(kc == 0), stop=(kc == NKc - 1))

            # software-pipelined by one chunk: attn@V for kc-1 is emitted after
            # scores(kc), so the exp(kc-1) wait never blocks independent score
            # matmuls behind it in the in-order PE stream
            prev = None
            for kc in range(NKc):
                sc = psA.tile([P, QB], F32, tag="ps", name=f"sc{h}")
                for ns in range(0, QB, NS):
                    nc.tensor.matmul(
                        sc[:, ns:ns + NS],
                        lhsT=KTh[:, kc * P:(kc + 1) * P],
                        rhs=QTh[:, q0 + ns:q0 + ns + NS],
                        start=True, stop=True)
                pT = pP.tile([P, QB], BF16, tag="pt", name=f"pt{h}")
                nc.scalar.activation(out=pT, in_=sc, func=AF.Exp, scale=0.125)
                if prev is not None:
                    attn_v(*prev)
                prev = (pT, kc)
            attn_v(*prev)
            ysb = ysbp.tile([HD, QB], BF16, tag="ysb", name=f"ysb{h}")
            nc.vector.tensor_copy(out=ysb, in_=yp[0:HD, :])
            lnden = lndp.tile([HD, QB], F32, tag="lnd", name=f"lnden{h}")
            nc.scalar.activation(out=lnden, in_=yp[HD:2 * HD, :], func=AF.Ln)
            return ysb, lnden

        def emit_pre(qb, h):
            """1/(sum v^2 + eps), broadcast -- independent of the attention
            output, so it runs alongside D1 and keeps D2's chain short."""
            q0 = qb * QB
            vth = VTh[h]
            vsq = stk.tile([HD, QB], BF16, tag="vsq")
            nc.vector.tensor_mul(vsq, vth[:, q0:q0 + QB], vth[:, q0:q0 + QB])
            d2B = psB.tile([HD, QB], F32, tag="yp", name="d2B")
            for ns in range(0, QB, NS):
                nc.tensor.matmul(d2B[:, ns:ns + NS], lhsT=ones64x64,
                                 rhs=vsq[:, ns:ns + NS], start=True, stop=True)
            lns = bcs2.tile([HD, QB], F32, tag="lns")
            nc.scalar.activation(out=lns, in_=d2B, func=AF.Ln, bias=epsv[0:HD, :])
            r2B = bcs2.tile([HD, QB], BF16, tag="r2b")
            nc.scalar.activation(out=r2B, in_=lns, func=AF.Exp, scale=-1.0)
            return r2B

        def heartbeat(dep):
            # tiny dependency-gated matmul (~60ns): spaces PE activity through
            # an otherwise PE-idle DVE chain so the clock-gate never sees a
            # fully-idle window and the tail keeps running at 2.4 GHz
            hb = psA.tile([HD, HD], F32, tag="ps", name="hb")
            nc.tensor.matmul(hb, lhsT=ones64x64, rhs=dep[:, 0:HD],
                             start=True, stop=True)

        def emit_d2(qb, h, ysb, lnden, r2B, hb=False):
            q0 = qb * QB
            vth = VTh[h]
            t_yv = stk.tile([HD, QB], BF16, tag="t_yv")
            nc.vector.tensor_mul(t_yv, ysb, vth[:, q0:q0 + QB])
            d1B = psB.tile([HD, QB], F32, tag="yp", name="d1B")
            for ns in range(0, QB, NS):
                nc.tensor.matmul(d1B[:, ns:ns + NS], lhsT=ones64x64,
                                 rhs=t_yv[:, ns:ns + NS], start=True, stop=True)

            betaB = bcs.tile([HD, QB], F32, tag="bet")
            nc.scalar.activation(out=betaB, in_=lnden, func=AF.Exp, scale=-1.0)

            aB = stk.tile([HD, QB], BF16, tag="ab")
            nc.vector.tensor_mul(aB, d1B[0:HD, :], r2B)
            if hb:
                heartbeat(aB)
            t2 = tps2.tile([HD, QB], BF16, tag="t2")
            nc.vector.tensor_mul(t2, vth[:, q0:q0 + QB], aB)
            u = tps.tile([HD, QB], BF16, tag="t1")
            nc.vector.tensor_sub(u, ysb, t2)
            if hb:
                heartbeat(u)
            nc.vector.tensor_mul(y_excl[h][:, q0:q0 + QB], u, betaB)

        def emit_e(qb, mt0=0, mt1=None):
            for mt in range(mt0, DM if mt1 is None else mt1):
                ps = psA.tile([P, QB], F32, tag="ps", name="ps_e")
                for h in range(HL):
                    lw = wo_bf[h][:, mt * P:(mt + 1) * P]
                    for ns in range(0, QB, NS):
                        nc.tensor.matmul(
                            ps[:, ns:ns + NS],
                            lhsT=lw,
                            rhs=y_excl[h][:, qb * QB + ns:qb * QB + ns + NS],
                            start=(h == 0), stop=(h == HL - 1))
                ostg = ostgp.tile([P, QB], F32, tag="ostg")
                nc.any.tensor_copy(out=ostg, in_=ps)
                nc.sync.dma_start(
                    out=outT_d.ap()[mt * P:(mt + 1) * P, qb * QB:(qb + 1) * QB],
                    in_=ostg)

        # ---- emission order: get the ACT-bound attention started early, then
        # feed the PE the remaining projection work to fill its dependency
        # cracks, so the PE never idles long enough to re-throttle. ----
        emit_qk(0)           # Q,K for heads 0,1
        emit_vprime()        # V' (needed by attn@V)
        saved = {}

        def d2_block(qb):
            # the 1/(sum v^2+eps) chains first: independent of the attention
            # output, they overlap the still-running D1s of the next q-block
            r2Bs = [emit_pre(qb, h) for h in range(HL)]
            for h in range(HL):
                emit_d2(qb, h, *saved[(qb, h)], r2Bs[h])

        saved[(0, 0)] = emit_d1(0, 0)
        emit_qk(1)           # Q,K heads 2,3 -- PE filler during D1 ACT stretches
        saved[(0, 1)] = emit_d1(0, 1)
        for h in range(HL):
            emit_vth(h)      # v^T per head -- more PE filler
        saved[(0, 2)] = emit_d1(0, 2)
        saved[(0, 3)] = emit_d1(0, 3)
        if NQ > 1:
            for qb in range(1, NQ):
                # spread the previous block's exclusive tails across the next
                # block's D1 stretches: each pair of chains hides behind ~20us
                # of scores instead of clustering into a PE-idle block
                saved[(qb, 0)] = emit_d1(qb, 0)
                pr0 = emit_pre(qb - 1, 0)
                pr1 = emit_pre(qb - 1, 1)
                emit_d2(qb - 1, 0, *saved[(qb - 1, 0)], pr0)
                emit_d2(qb - 1, 1, *saved[(qb - 1, 1)], pr1)
                saved[(qb, 1)] = emit_d1(qb, 1)
                pr2 = emit_pre(qb - 1, 2)
                pr3 = emit_pre(qb - 1, 3)
                emit_d2(qb - 1, 2, *saved[(qb - 1, 2)], pr2)
                emit_d2(qb - 1, 3, *saved[(qb - 1, 3)], pr3)
                saved[(qb, 2)] = emit_d1(qb, 2)
                # this q-block's first two exclusive tails run mid-kernel,
                # hidden behind the remaining D1 scores; only two chains are
                # left exposed at the very end
                r2b0 = emit_pre(qb, 0)
                r2b1 = emit_pre(qb, 1)
                emit_d2(qb, 0, *saved[(qb, 0)], r2b0)
                emit_d2(qb, 1, *saved[(qb, 1)], r2b1)
                saved[(qb, 3)] = emit_d1(qb, 3)
                emit_e(qb - 1)
            qL = NQ - 1
            r2b2 = emit_pre(qL, 2)
            r2b3 = emit_pre(qL, 3)
            emit_d2(qL, 2, *saved[(qL, 2)], r2b2, hb=True)
            emit_d2(qL, 3, *saved[(qL, 3)], r2b3, hb=True)
            emit_e(qL)
        else:
            d2_block(0)
            emit_e(0)

    nc.finalize()
    return nc


def shard_inputs(x, Wq, bq, Wk, bk, Wv, bv, Wo, bo, n_cores=N_CORES):
    """Full inputs -> per-core input maps (host-side transpose/slice/reshape)."""
    H = Wq.shape[1]
    cores_per_batch = n_cores // x.shape[0]
    hl = H // cores_per_batch
    in_maps = []
    for c in range(n_cores):
        b = c // cores_per_batch
        h0 = (c % cores_per_batch) * hl
        bf = ml_dtypes.bfloat16
        m = {
            "xT": np.ascontiguousarray(x[b].T).astype(bf),
            "wq": np.ascontiguousarray(Wq[:, h0:h0 + hl, :].reshape(Wq.shape[0], -1)).astype(bf),
            "wk": np.ascontiguousarray(Wk[:, h0:h0 + hl, :].reshape(Wk.shape[0], -1)).astype(bf),
            "wv": np.ascontiguousarray(Wv[:, h0:h0 + hl, :].reshape(Wv.shape[0], -1)).astype(bf),
            "wo": np.ascontiguousarray(Wo[h0:h0 + hl].reshape(-1, Wo.shape[2])).astype(bf),
        }
        if _use_bias(bq, bk, bv):
            m["bq"] = np.ascontiguousarray(bq[h0:h0 + hl].reshape(1, -1)).astype(np.float32)
            m["bk"] = np.ascontiguousarray(bk[h0:h0 + hl].reshape(1, -1)).astype(np.float32)
            m["bv"] = np.ascontiguousarray(bv[h0:h0 + hl].reshape(1, -1)).astype(np.float32)
        in_maps.append(m)
    return in_maps


def _use_bias(bq, bk, bv):
    return bool(np.any(bq) or np.any(bk) or np.any(bv))


_ACT_ROOT_READY = False


def _ensure_act_root():
    """Point walrus at an act-table root whose only set is
    natural_log_exp_and_others, so exp and ln share one ACT table set and the
    kernel never pays mid-stream ACT_TABLE_LOADs (which stall the PE long
    enough to re-throttle its clock)."""
    global _ACT_ROOT_READY
    if _ACT_ROOT_READY or os.environ.get("BASS_ACT_ROOT_JSON_PATH"):
        _ACT_ROOT_READY = True
        return
    import json
    import tempfile
    from neuronxcc.driver.Job import Job
    from neuronxcc.driver.jobs.support.FindActInfo import findActInfoFile

    orig = findActInfoFile(Job.getPackageDir(), "gen3")
    with open(orig) as f:
        info = json.load(f)
    keep = [e for e in info["act_func_sets"]
            if e["name"] == "natural_log_exp_and_others"]
    if not keep:  # unexpected layout -- fall back to stock tables
        _ACT_ROOT_READY = True
        return
    root = tempfile.mkdtemp(prefix="act_root_")
    src_dir = os.path.dirname(orig)
    for fn in os.listdir(src_dir):
        if fn != "act_info.json":
            os.symlink(os.path.join(src_dir, fn), os.path.join(root, fn))
    info["act_func_sets"] = keep
    with open(os.path.join(root, "act_info.json"), "w") as f:
        json.dump(info, f)
    os.environ["BASS_ACT_ROOT_JSON_PATH"] = os.path.join(root, "act_info.json")

    # Bacc preplaces InstLoadActFuncSet using concourse.hw_specs tables (it
    # reads the stock act_info directly); keep its set-id numbering in sync
    # with the custom single-set root.
    import concourse.hw_specs as hw_specs
    import concourse.bacc as bacc_mod
    _orig_tables = hw_specs.get_activation_tables

    def _single_set_tables(module_arch):
        tables = _orig_tables(module_arch)
        if "natural_log_exp_and_others" in tables:
            return {"natural_log_exp_and_others": tables["natural_log_exp_and_others"]}
        return tables

    hw_specs.get_activation_tables = _single_set_tables
    bacc_mod.get_activation_tables = _single_set_tables
    _ACT_ROOT_READY = True


_NC_CACHE = {}


def _get_nc(use_bias):
    if use_bias not in _NC_CACHE:
        _NC_CACHE[use_bias] = build_nc(use_bias=use_bias)
    return _NC_CACHE[use_bias]


def run_sharded(inputs, trace=False, trace_cores=None):
    """Run the SPMD kernel; returns (full_output, BassKernelResults)."""
    x, bo = inputs["x"], inputs["bo"]
    use_bias = _use_bias(inputs["bq"], inputs["bk"], inputs["bv"])
    _ensure_act_root()
    nc = _get_nc(use_bias)
    in_maps = shard_inputs(**inputs)
    res = bass_utils.run_bass_kernel_spmd(
        nc, in_maps, core_ids=list(range(N_CORES)),
        trace=trace, trace_cores=trace_cores)
    cores_per_batch = N_CORES // x.shape[0]
    out = np.empty_like(x)
    for b in range(x.shape[0]):
        acc = np.zeros((x.shape[2], x.shape[1]), np.float32)
        for c in range(b * cores_per_batch, (b + 1) * cores_per_batch):
            acc += res.results[c]["outT"]
        out[b] = acc.T + bo[None, :]
    return out, res


def kernel(**inputs):
    out, _ = run_sharded(inputs)
    return out



# revision 15
# speedup vs baseline: 1.2677x; 1.2677x over previous
"""Multi-head attention with exclusive post-processing, sharded over 8 trn2 cores.

Sharding: data-parallel over batch (2) x tensor-parallel over heads (16 -> 4/core).
Each core computes a partial transposed output [D, S] (fp16) for its batch from
its 4 heads; the host sums the 4 partials per batch, transposes back, adds bo.

Per-core design (v2 -- pair-fused, ACT-paced):
  Heads are processed in PAIRS sharing the 128-partition dim (even head at
  partitions 0-63, odd head at 64-127):
    QT/KT/VT [128, S]  per pair (feature-major, bf16)
    scores: both heads' score matmuls interleave as concurrent PE row-tiles
      (0,0)/(64,0) into one [128, 2*QC] PSUM tile (h0 cols | h1 cols)
    ONE exp (FD=2*QC) covers the pair -> pT [128, 2*QC] bf16
    attnV per head: lhsT = [V|ones] position-major -> yp [128, QC]
      (rows 0-63 Y, 64-127 softmax denominator)
    exclusive tail runs pair-fused on [128, QC] tiles: ysb relocation copy
      puts the odd head's Y at partitions 64-127; Ln/exp reciprocals
      (exp(-ln x), one ACT table set) and DVE muls cover both heads at the
      same FD cost as one.
    out-projection fuses the pair as a single K=128 contraction
      (Wo rows h0|h1 stacked = sum over both heads for free).
  V' ([V|ones] position-major) accumulates kc-outer DURING the xT input DMA
  stream (16 half-bank PSUM accumulators); inputs split across the sync and
  scalar hardware DMA queues; output is fp16 on the sync queue.
  PSUM: scores 2x[128,1024] (4 banks) + yp 2x[128,512] (2) + filler 2x[128,512]
  (2).  QK tile1 / VT tile1 / out-proj run as fillers inside the ACT-paced
  attention windows via the filler pool.
"""

import os
from contextlib import ExitStack

import ml_dtypes
import numpy as np

import concourse.bass as bass
import concourse.mybir as mybir
import concourse.tile as tile
from concourse import bacc, bass_utils

F32 = mybir.dt.float32
BF16 = mybir.dt.bfloat16
F16 = mybir.dt.float16
AF = mybir.ActivationFunctionType

B, S_FULL, D_FULL, H_FULL = 2, 2048, 1024, 16
HD = 64
N_CORES = 8
HEADS_PER_CORE = H_FULL * B // N_CORES  # 4


def build_nc(S=S_FULL, D=D_FULL, use_bias=False, debug=False):
    P = 128
    HL = HEADS_PER_CORE          # 4 local heads = 2 pairs
    NP = HL // 2                 # pairs (2)
    nH = HL * HD                 # 256
    KC = D // P                  # x contraction chunks (8)
    NKc = S // P                 # key chunks (16)
    QC = 512                     # q chunk (scores pair tile = [128, 2*QC])
    NQ = S // QC                 # 4
    DM = D // P                  # out-proj M tiles (8)

    assert not use_bias, "bias path not implemented (reference biases are zero)"
    _ensure_act_root()
    nc = bacc.Bacc(None, target_bir_lowering=False)

    xT_d = nc.dram_tensor("xT", [D, S], BF16, kind="ExternalInput")
    wq_d = nc.dram_tensor("wq", [D, nH], BF16, kind="ExternalInput")
    wk_d = nc.dram_tensor("wk", [D, nH], BF16, kind="ExternalInput")
    wv_d = nc.dram_tensor("wv", [D, nH], BF16, kind="ExternalInput")
    wo_d = nc.dram_tensor("wo", [nH, D], BF16, kind="ExternalInput")
    outT_d = nc.dram_tensor("outT", [D, S], F16, kind="ExternalOutput")
    dbg = {}
    if debug:
        for name, shape, dt in (("dQT0", [128, S], BF16), ("dKT0", [128, S], BF16),
                                ("dQT1", [128, S], BF16), ("dKT1", [128, S], BF16),
                                ("dVT0", [128, S], BF16), ("dVT1", [128, S], BF16),
                                ("dVP", [128, 16 * 4 * 128], BF16),
                                ("dYX0", [128, S], BF16), ("dYX1", [128, S], BF16),
                                ("dYSB", [128, 512], BF16),
                                ("dLND", [128, 512], F32),
                                ("dPT", [128, 1024], BF16)):
            dbg[name] = nc.dram_tensor(name, shape, dt, kind="ExternalOutput")

    with tile.TileContext(nc) as tc, ExitStack() as ctx:
        consts = ctx.enter_context(tc.tile_pool(name="consts", bufs=1))
        psSC = ctx.enter_context(tc.tile_pool(name="psSC", bufs=2, space="PSUM"))
        psYP = ctx.enter_context(tc.tile_pool(name="psYP", bufs=2, space="PSUM"))
        psFL = ctx.enter_context(tc.tile_pool(name="psFL", bufs=2, space="PSUM"))
        pP = ctx.enter_context(tc.tile_pool(name="pP", bufs=4))
        ostgp = ctx.enter_context(tc.tile_pool(name="ostgp", bufs=3))
        stk = ctx.enter_context(tc.tile_pool(name="stk", bufs=2))
        stk2 = ctx.enter_context(tc.tile_pool(name="stk2", bufs=2))
        ysbp = ctx.enter_context(tc.tile_pool(name="ysbp", bufs=3))
        lndp = ctx.enter_context(tc.tile_pool(name="lndp", bufs=3))
        bcp = ctx.enter_context(tc.tile_pool(name="bcp", bufs=3))

        # ---- ACT table preload (single exp+ln set; see _ensure_act_root) ----
        smallc = consts.tile([P, 33], F32, tag="smallc")
        warm = smallc[0:1, 1:33]
        nc.vector.memset(warm, 1.0)
        nc.scalar.activation(out=warm, in_=warm, func=AF.Exp)
        nc.scalar.activation(out=warm, in_=warm, func=AF.Ln)
        epsv = smallc[:, 0:1]
        nc.vector.memset(epsv, 1e-12)

        ones128 = consts.tile([P, HD], BF16, tag="ones128")
        nc.vector.memset(ones128, 1.0)

        # ---- input DMAs split across the two HW queues ----
        # scalar queue: wv first (gates the streamed V' accumulation), xT odd
        # sync queue:   xT even, then wq/wk/wo
        wv_sb = [consts.tile([P, nH], BF16, tag=f"wv{kc}", name=f"wv{kc}") for kc in range(KC)]
        for kc in range(KC):
            nc.scalar.dma_start(out=wv_sb[kc],
                                in_=wv_d.ap()[kc * P:(kc + 1) * P, :])
        xT_sb = [consts.tile([P, S], BF16, tag=f"xT{kc}", name=f"xT{kc}") for kc in range(KC)]
        for kc in range(KC):
            eng = nc.sync if kc % 2 == 0 else nc.scalar
            eng.dma_start(out=xT_sb[kc], in_=xT_d.ap()[kc * P:(kc + 1) * P, :])
        wq_sb = [consts.tile([P, nH], BF16, tag=f"wq{kc}", name=f"wq{kc}") for kc in range(KC)]
        wk_sb = [consts.tile([P, nH], BF16, tag=f"wk{kc}", name=f"wk{kc}") for kc in range(KC)]
        for kc in range(KC):
            nc.sync.dma_start(out=wq_sb[kc], in_=wq_d.ap()[kc * P:(kc + 1) * P, :])
            nc.sync.dma_start(out=wk_sb[kc], in_=wk_d.ap()[kc * P:(kc + 1) * P, :])
        wo_sb = [consts.tile([P, D], BF16, tag=f"wo{p}", name=f"wo{p}") for p in range(NP)]
        for p in range(NP):
            nc.sync.dma_start(out=wo_sb[p], in_=wo_d.ap()[p * P:(p + 1) * P, :])

        # ---- phase V': [V|ones] position-major, kc-outer during the xT
        # stream.  16 half-bank accumulators [128, nH] live across all of
        # PSUM; each xT chunk contributes one matmul per qt. ----
        vprime = consts.tile([P, NKc, HL, 2 * HD], BF16, tag="vprime")
        nc.vector.memset(vprime[:, :, :, HD:2 * HD], 1.0)
        # 8 bank-aligned [128, nH] accumulators per round (PSUM matmul writes
        # must start on a 2KB bank boundary), two rounds of 8 qt chunks;
        # round 0 overlaps the xT DMA stream, round 1 re-reads SBUF xT.
        for rnd in range(2):
            acc_map = []
            for pool, tag, n_acc in ((psSC, "sc", 2), (psSC, "sc", 2),
                                     (psYP, "yp", 1), (psYP, "yp", 1),
                                     (psFL, "fl", 1), (psFL, "fl", 1)):
                t = pool.tile([P, n_acc * 2 * nH], F32, tag=tag, name="vacc")
                for j in range(n_acc):
                    acc_map.append(t[:, j * 2 * nH:j * 2 * nH + nH])
            assert len(acc_map) == NKc // 2
            for kc in range(KC):
                for j, qt in enumerate(range(rnd * 8, rnd * 8 + 8)):
                    nc.tensor.matmul(
                        acc_map[j],
                        lhsT=xT_sb[kc][:, qt * P:(qt + 1) * P],
                        rhs=wv_sb[kc],
                        start=(kc == 0), stop=(kc == KC - 1))
            for j, qt in enumerate(range(rnd * 8, rnd * 8 + 8)):
                nc.vector.tensor_copy(
                    out=vprime[:, qt, :, 0:HD],
                    in_=acc_map[j].rearrange("p (h d) -> p h d", h=HL))

        # ---- QK tile0 (pair 0) ----
        QT = [consts.tile([P, S], BF16, tag=f"QT{p}", name=f"QT{p}") for p in range(NP)]
        KT = [consts.tile([P, S], BF16, tag=f"KT{p}", name=f"KT{p}") for p in range(NP)]
        VT = [consts.tile([P, S], BF16, tag=f"VT{p}", name=f"VT{p}") for p in range(NP)]

        def emit_proj_chunk(w_sb, dst, p, q0, qw, pool=None, tag="fl"):
            """dst[p][:, q0:q0+qw] = (W pair-slice).T @ xT  (accumulate KC)."""
            pool = pool or psFL
            ps = pool.tile([P, qw], F32, tag=tag, name="ps_proj")
            for kc in range(KC):
                for ns in range(0, qw, 512):
                    nc.tensor.matmul(
                        ps[:, ns:ns + 512],
                        lhsT=w_sb[kc][:, p * P:(p + 1) * P],
                        rhs=xT_sb[kc][:, q0 + ns:q0 + ns + 512],
                        start=(kc == 0), stop=(kc == KC - 1))
            nc.vector.tensor_copy(out=dst[p][:, q0:q0 + qw], in_=ps)

        for q0 in range(0, S, 1024):
            emit_proj_chunk(wq_sb, QT, 0, q0, 1024, pool=psSC, tag="sc")
            emit_proj_chunk(wk_sb, KT, 0, q0, 1024, pool=psSC, tag="sc")
        for q0 in range(0, S, 1024):
            emit_proj_chunk(wv_sb, VT, 0, q0, 1024, pool=psSC, tag="sc")

        # ---- filler queue: emitted inside the ACT-paced attention loops ----
        fillers = []

        def fill_qk1():
            for q0 in range(0, S, QC):
                fillers.append(lambda q0=q0: emit_proj_chunk(wq_sb, QT, 1, q0, QC))
                fillers.append(lambda q0=q0: emit_proj_chunk(wk_sb, KT, 1, q0, QC))

        def fill_vt1():
            for q0 in range(0, S, QC):
                fillers.append(lambda q0=q0: emit_proj_chunk(wv_sb, VT, 1, q0, QC))

        fill_qk1()
        fill_vt1()

        y_excl = [consts.tile([P, S], BF16, tag=f"yx{p}", name=f"yx{p}") for p in range(NP)]

        # ---- D1 pair loop ----
        def emit_d1(p, qc):
            q0 = qc * QC
            yp0 = psYP.tile([P, QC], F32, tag="yp", name=f"yp0_{p}")
            yp1 = psYP.tile([P, QC], F32, tag="yp", name=f"yp1_{p}")

            def attn_v(pT, kc):
                nc.tensor.matmul(
                    yp0, lhsT=vprime[:, kc, 2 * p, :], rhs=pT[:, 0:QC],
                    start=(kc == 0), stop=(kc == NKc - 1))
                nc.tensor.matmul(
                    yp1, lhsT=vprime[:, kc, 2 * p + 1, :], rhs=pT[:, QC:2 * QC],
                    start=(kc == 0), stop=(kc == NKc - 1))

            prev = None
            for kc in range(NKc):
                sc = psSC.tile([P, 2 * QC], F32, tag="sc", name=f"sc{p}")
                nc.tensor.matmul(
                    sc[:, 0:QC],
                    lhsT=KT[p][0:HD, kc * P:(kc + 1) * P],
                    rhs=QT[p][0:HD, q0:q0 + QC], start=True, stop=True)
                nc.tensor.matmul(
                    sc[:, QC:2 * QC],
                    lhsT=KT[p][HD:P, kc * P:(kc + 1) * P],
                    rhs=QT[p][HD:P, q0:q0 + QC], start=True, stop=True)
                pT = pP.tile([P, 2 * QC], BF16, tag="pt", name=f"pt{p}")
                nc.scalar.activation(out=pT, in_=sc, func=AF.Exp, scale=0.125)
                if prev is not None:
                    attn_v(*prev)
                prev = (pT, kc)
                if kc % 5 == 4 and fillers:
                    fillers.pop(0)()
            attn_v(*prev)

            # extraction: ysb pair (odd head relocated to partitions 64-127),
            # lnden pair
            ysb = ysbp.tile([P, QC], BF16, tag="ysb", name=f"ysb{p}")
            nc.vector.tensor_copy(out=ysb[0:HD, :], in_=yp0[0:HD, :])
            nc.vector.tensor_copy(out=ysb[HD:P, :], in_=yp1[0:HD, :])
            lnden = lndp.tile([P, QC], F32, tag="lnd", name=f"lnden{p}")
            nc.scalar.activation(out=lnden[0:HD, :], in_=yp0[HD:P, :], func=AF.Ln)
            nc.scalar.activation(out=lnden[HD:P, :], in_=yp1[HD:P, :], func=AF.Ln)
            return ysb, lnden, prev[0]

        def heartbeat(dep):
            hb = psFL.tile([HD, HD], F32, tag="fl", name="hb")
            nc.tensor.matmul(hb, lhsT=ones128[0:HD, :], rhs=dep[0:HD, 0:HD],
                             start=True, stop=True)

        # ---- exclusive tail, pair-fused on [128, QC] ----
        def emit_d2(p, qc, ysb, lnden, hb=False):
            q0 = qc * QC
            vth = VT[p][:, q0:q0 + QC]
            # r2 = 1/(sum_hd v^2 + eps) per head, broadcast over 64 partitions
            vsq = stk.tile([P, QC], BF16, tag="vsq")
            nc.vector.tensor_mul(vsq, vth, vth)
            d2B = psFL.tile([P, QC], F32, tag="fl", name="d2B")
            nc.tensor.matmul(d2B[0:HD, :], lhsT=ones128[0:HD, :],
                             rhs=vsq[0:HD, :], start=True, stop=True)
            nc.tensor.matmul(d2B[HD:P, :], lhsT=ones128[HD:P, :],
                             rhs=vsq[HD:P, :], start=True, stop=True)
            lns = bcp.tile([P, QC], F32, tag="lns")
            nc.scalar.activation(out=lns, in_=d2B, func=AF.Ln, bias=epsv)
            r2c = bcp.tile([P, QC], BF16, tag="r2c")
            nc.scalar.activation(out=r2c, in_=lns, func=AF.Exp, scale=-1.0)

            t_yv = stk.tile([P, QC], BF16, tag="t_yv")
            nc.vector.tensor_mul(t_yv, ysb, vth)
            d1B = psFL.tile([P, QC], F32, tag="fl", name="d1B")
            nc.tensor.matmul(d1B[0:HD, :], lhsT=ones128[0:HD, :],
                             rhs=t_yv[0:HD, :], start=True, stop=True)
            nc.tensor.matmul(d1B[HD:P, :], lhsT=ones128[HD:P, :],
                             rhs=t_yv[HD:P, :], start=True, stop=True)

            beta = bcp.tile([P, QC], BF16, tag="bet")
            nc.scalar.activation(out=beta, in_=lnden, func=AF.Exp, scale=-1.0)

            aB = stk2.tile([P, QC], BF16, tag="ab")
            nc.vector.tensor_mul(aB, d1B, r2c)
            if hb:
                heartbeat(aB)
            t2 = stk2.tile([P, QC], BF16, tag="t2")
            nc.vector.tensor_mul(t2, vth, aB)
            u = stk.tile([P, QC], BF16, tag="u")
            nc.vector.tensor_sub(u, ysb, t2)
            if hb:
                heartbeat(u)
            nc.vector.tensor_mul(y_excl[p][:, q0:q0 + QC], u, beta)

        # ---- out-projection for one (mt, qc): K=128 pair-fused ----
        def emit_e_chunk(mt, qc):
            q0 = qc * QC
            ps = psFL.tile([P, QC], F32, tag="fl", name="ps_e")
            for p in range(NP):
                nc.tensor.matmul(
                    ps, lhsT=wo_sb[p][:, mt * P:(mt + 1) * P],
                    rhs=y_excl[p][:, q0:q0 + QC],
                    start=(p == 0), stop=(p == NP - 1))
            ostg = ostgp.tile([P, QC], F16, tag="ostg")
            nc.vector.tensor_copy(out=ostg, in_=ps)
            nc.sync.dma_start(
                out=outT_d.ap()[mt * P:(mt + 1) * P, q0:q0 + QC], in_=ostg)

        def fill_e(qc):
            for mt0 in range(0, DM, 2):
                def f(mt0=mt0, qc=qc):
                    emit_e_chunk(mt0, qc)
                    emit_e_chunk(mt0 + 1, qc)
                fillers.append(f)

        # ---- schedule: all pair-0 loops, then pair-1 loops; out-proj of qc
        # becomes available after pair-1's tail for that qc ----
        plan = [(0, qc) for qc in range(NQ)] + [(1, qc) for qc in range(NQ)]
        n_loops = len(plan)
        for i, (p, qc) in enumerate(plan):
            saved = emit_d1(p, qc)
            if debug and i == 0:
                nc.sync.dma_start(out=dbg["dYSB"].ap(), in_=saved[0])
                nc.sync.dma_start(out=dbg["dLND"].ap(), in_=saved[1])
                nc.sync.dma_start(out=dbg["dPT"].ap(), in_=saved[2])
            last = (i == n_loops - 1)
            emit_d2(p, qc, *saved[:2], hb=last)
            if p == 1:
                fill_e(qc)
            if last:
                while fillers:
                    fillers.pop(0)()
        if debug:
            for nm, t in (("dQT0", QT[0]), ("dKT0", KT[0]), ("dQT1", QT[1]),
                          ("dKT1", KT[1]), ("dVT0", VT[0]), ("dVT1", VT[1]),
                          ("dYX0", y_excl[0]), ("dYX1", y_excl[1])):
                nc.sync.dma_start(out=dbg[nm].ap(), in_=t)
            nc.sync.dma_start(
                out=dbg["dVP"].ap(),
                in_=vprime.rearrange("p a b c -> p (a b c)"))

    nc.finalize()
    return nc


def shard_inputs(x, Wq, bq, Wk, bk, Wv, bv, Wo, bo, n_cores=N_CORES):
    """Full inputs -> per-core input maps (host-side transpose/slice/reshape)."""
    H = Wq.shape[1]
    cores_per_batch = n_cores // x.shape[0]
    hl = H // cores_per_batch
    in_maps = []
    for c in range(n_cores):
        b = c // cores_per_batch
        h0 = (c % cores_per_batch) * hl
        bf = ml_dtypes.bfloat16
        m = {
            "xT": np.ascontiguousarray(x[b].T).astype(bf),
            "wq": np.ascontiguousarray(Wq[:, h0:h0 + hl, :].reshape(Wq.shape[0], -1)).astype(bf),
            "wk": np.ascontiguousarray(Wk[:, h0:h0 + hl, :].reshape(Wk.shape[0], -1)).astype(bf),
            "wv": np.ascontiguousarray(Wv[:, h0:h0 + hl, :].reshape(Wv.shape[0], -1)).astype(bf),
            "wo": np.ascontiguousarray(Wo[h0:h0 + hl].reshape(-1, Wo.shape[2])).astype(bf),
        }
        if _use_bias(bq, bk, bv):
            m["bq"] = np.ascontiguousarray(bq[h0:h0 + hl].reshape(1, -1)).astype(np.float32)
            m["bk"] = np.ascontiguousarray(bk[h0:h0 + hl].reshape(1, -1)).astype(np.float32)
            m["bv"] = np.ascontiguousarray(bv[h0:h0 + hl].reshape(1, -1)).astype(np.float32)
        in_maps.append(m)
    return in_maps


def _use_bias(bq, bk, bv):
    return bool(np.any(bq) or np.any(bk) or np.any(bv))


_ACT_ROOT_READY = False


def _ensure_act_root():
    """Point walrus at an act-table root whose only set is
    natural_log_exp_and_others, so exp and ln share one ACT table set and the
    kernel never pays mid-stream ACT_TABLE_LOADs."""
    global _ACT_ROOT_READY
    if _ACT_ROOT_READY or os.environ.get("BASS_ACT_ROOT_JSON_PATH"):
        _ACT_ROOT_READY = True
        return
    import json
    import tempfile
    from neuronxcc.driver.Job import Job
    from neuronxcc.driver.jobs.support.FindActInfo import findActInfoFile

    orig = findActInfoFile(Job.getPackageDir(), "gen3")
    with open(orig) as f:
        info = json.load(f)
    keep = [e for e in info["act_func_sets"]
            if e["name"] == "natural_log_exp_and_others"]
    if not keep:
        _ACT_ROOT_READY = True
        return
    root = tempfile.mkdtemp(prefix="act_root_")
    src_dir = os.path.dirname(orig)
    for fn in os.listdir(src_dir):
        if fn != "act_info.json":
            os.symlink(os.path.join(src_dir, fn), os.path.join(root, fn))
    info["act_func_sets"] = keep
    with open(os.path.join(root, "act_info.json"), "w") as f:
        json.dump(info, f)
    os.environ["BASS_ACT_ROOT_JSON_PATH"] = os.path.join(root, "act_info.json")

    import concourse.hw_specs as hw_specs
    import concourse.bacc as bacc_mod
    _orig_tables = hw_specs.get_activation_tables

    def _single_set_tables(module_arch):
        tables = _orig_tables(module_arch)
        if "natural_log_exp_and_others" in tables:
            return {"natural_log_exp_and_others": tables["natural_log_exp_and_others"]}
        return tables

    hw_specs.get_activation_tables = _single_set_tables
    bacc_mod.get_activation_tables = _single_set_tables
    _ACT_ROOT_READY = True


_NC_CACHE = {}


def _get_nc(use_bias):
    if use_bias not in _NC_CACHE:
        _NC_CACHE[use_bias] = build_nc(use_bias=use_bias)
    return _NC_CACHE[use_bias]


def run_sharded(inputs, trace=False, trace_cores=None):
    """Run the SPMD kernel; returns (full_output, BassKernelResults)."""
    x, bo = inputs["x"], inputs["bo"]
    use_bias = _use_bias(inputs["bq"], inputs["bk"], inputs["bv"])
    _ensure_act_root()
    nc = _get_nc(use_bias)
    in_maps = shard_inputs(**inputs)
    res = bass_utils.run_bass_kernel_spmd(
        nc, in_maps, core_ids=list(range(N_CORES)),
        trace=trace, trace_cores=trace_cores)
    cores_per_batch = N_CORES // x.shape[0]
    out = np.empty_like(x)
    for b in range(x.shape[0]):
        acc = np.zeros((x.shape[2], x.shape[1]), np.float32)
        for c in range(b * cores_per_batch, (b + 1) * cores_per_batch):
            acc += res.results[c]["outT"].astype(np.float32)
        out[b] = acc.T + bo[None, :]
    return out, res


def kernel(**inputs):
    out, _ = run_sharded(inputs)
    return out
